# revision 4
# baseline (speedup 1.0000x reference)
"""Trainium2 Bass kernel for nn_LlamaAttention (T=2048, HID=4096, HQ=32, HKV=8, D=128).

Tensor-parallel over heads across 8 NeuronCores: core c owns q-heads 4c..4c+3 and
kv-head c (GQA group size 4 == heads-per-core, so attention is fully core-local).
Wo is row-sharded; each core computes a partial [HID, T] output (transposed, bf16)
and the host sums the 8 partials. No device collectives.

v3 vs v2:
 - HAM warm-up: ~12 dummy matmuls on memset data right after the preamble, so the
   PE clock is at 2.4 GHz (not 1.2) when the first real DMA lands (~-2.4us)
 - per-chunk tiles for q/kT/vT/v/attn: kills the tile-granular false deps that
   made the first B matmuls wait on the last A-chunk PSUM drains (~-2.6us incl.
   the HAM re-throttle the bubble caused)
 - last A chunk: k=31 emits chains in m-order [0,3,1,2,4,5] and the drains split
   scalar/vector ordered to match B's PSUM-bank reuse order, so B's first
   score/AV matmuls find their banks free
 - v transposes as REGULAR matmuls vs identity (not transpose-mode): ~81ns vs
   ~311ns each, and they count as PE-busy for HAM (~-3us)
 - softmax denominator via gpsimd partition_all_reduce instead of ones-matmuls
   on the PE (~-4us PE); the last-processed chunk (ci=0, tail-critical) keeps
   the ones-matmul path
 - final phase-C output DMAs at 2-mo granularity so the last transfer after the
   last matmul is 256KB, not 512KB+queue (~-3us tail)

Self-contained: hardcodes all shapes; builds the Bass kernel once per process.
"""
import numpy as np
import ml_dtypes

T, HID, HQ, HKV, D = 2048, 4096, 32, 8, 128
NCORES = 8
HPC = HQ // NCORES            # 4 q heads per core
QW = HPC * D                  # 512 q columns per core
MW = QW + 2 * D               # 768 qkv columns per core
KO = HID // 128               # 32 k-tiles
ACH = 512                     # phase A t-chunk width (PSUM bank)
NCHA = T // ACH               # 4
CH = 512                      # attention tq chunk width (PSUM bank)
MO = HID // 128               # 32 output row-tiles
SCALING = float(D) ** -0.5
BF16NP = ml_dtypes.bfloat16

_CACHE = {}


def _build_nc():
    import concourse.mybir as mybir
    import concourse.tile as tile
    from concourse import bacc, bass_isa
    from contextlib import ExitStack

    F32 = mybir.dt.float32
    BF = mybir.dt.bfloat16
    AF = mybir.ActivationFunctionType

    nc = bacc.Bacc("TRN2", target_bir_lowering=False, debug=False,
                   dynamic_dma_scratch_size=2048)

    # pre-swizzled inputs (see prep_in_maps)
    hidp = nc.dram_tensor("hidp", [128, NCHA * KO * ACH], BF, kind="ExternalInput")
    wqkvp = nc.dram_tensor("wqkvp", [128, KO * MW], BF, kind="ExternalInput")
    wop = nc.dram_tensor("wop", [128, HPC * MO * 128], BF, kind="ExternalInput")
    cosT = nc.dram_tensor("cosT", [D, T], BF, kind="ExternalInput")
    sinT = nc.dram_tensor("sinT", [D, T], BF, kind="ExternalInput")
    onesd = nc.dram_tensor("onesd", [128, 128], BF, kind="ExternalInput")
    identd = nc.dram_tensor("identd", [128, 128], BF, kind="ExternalInput")
    masksp = nc.dram_tensor("masksp", [128, 4 * CH], BF, kind="ExternalInput")
    outT = nc.dram_tensor("outT_p", [128, MO, T], BF, kind="ExternalOutput")

    with tile.TileContext(nc) as tc, ExitStack() as ctx:
        # ---- HAM warm-up: dummy matmuls on memset data while the first real
        # DMAs are still in flight. The PE's clock gate needs ~3.4us of
        # sustained activity to go 4/8 -> 8/8; this burns the otherwise-idle
        # 6..12us startup window so real compute starts at full clock.
        warm_pool = ctx.enter_context(tc.tile_pool(name="warm", bufs=1))
        gsb = warm_pool.tile([128, 512], BF)

        consts = ctx.enter_context(tc.tile_pool(name="consts", bufs=1))
        ones_sb = consts.tile([128, 128], BF)
        ident = consts.tile([128, 128], BF)
        mask_sb = consts.tile([128, 4 * CH], BF)
        cos_sb = consts.tile([128, T], BF)
        sin_sb = consts.tile([128, T], BF)

        qkv_pool = ctx.enter_context(tc.tile_pool(name="qkvT", bufs=1))
        # qk_c[m][ch]: m=0..3 q heads, m=4 kT, m=5 vT -- per-chunk [128, 512]
        qk_c = [[qkv_pool.tile([128, ACH], BF, name=f"qk{m}_{ch}")
                 for ch in range(NCHA)] for m in range(6)]
        # v_c[ch][s, jj, d]: per-chunk transposed V (4 j-tiles per chunk)
        v_c = [qkv_pool.tile([128, 4, 128], BF, name=f"v_{ch}")
               for ch in range(NCHA)]

        wopool = ctx.enter_context(tc.tile_pool(name="wo", bufs=1))
        wo_sb = wopool.tile([128, HPC * MO * 128], BF)

        with tc.tile_pool(name="psWm", bufs=1, space="PSUM") as psWm:
            nc.gpsimd.memset(gsb[:], 0.0)
            pw = psWm.tile([128, 512], F32)
            for _ in range(12):
                nc.tensor.matmul(pw[:], gsb[:, :128], gsb[:],
                                 start=True, stop=True)

        # ---- Phase A: qkvT = wqkv.T @ hidT, one 32-step PSUM chain per (ch,m) --
        with tc.tile_pool(name="w", bufs=1) as wp, \
             tc.tile_pool(name="hid", bufs=3) as hp, \
             tc.tile_pool(name="ropetmp", bufs=2) as tmp_pool, \
             tc.tile_pool(name="psA", bufs=6, space="PSUM") as psA, \
             tc.tile_pool(name="psT", bufs=2, space="PSUM") as psT:
            w_sb = wp.tile([128, KO * MW], BF, name="w_sb")

            def wdma(k0, k1):
                nc.sync.dma_start(w_sb[:, k0 * MW:k1 * MW], wqkvp[:, k0 * MW:k1 * MW])

            HHALF = 16 * ACH  # half-chunk: 16 k-tiles

            def hdma(ht, ch, half, kk0, kk1):
                # local k-tile range [kk0,kk1) of half `half` of chunk ch
                base = (ch * KO + half * 16) * ACH
                nc.sync.dma_start(ht[:, kk0 * ACH:kk1 * ACH],
                                  hidp[:, base + kk0 * ACH:base + kk1 * ACH])

            def halloc(ch, half):
                return hp.tile([128, HHALF], BF, tag="hid", name=f"h{ch}_{half}")

            # startup: k0-2 of w AND hid land before the chain's first matmul
            # (which waits on w0+h0): a ~2-3 k-tile buffer absorbs DMA jitter
            # on a stream that otherwise runs at ~91% of delivery bandwidth.
            h0a = halloc(0, 0)
            wdma(0, 1); hdma(h0a, 0, 0, 0, 1)
            wdma(1, 3); hdma(h0a, 0, 0, 1, 3)
            for k in range(3, 8):
                wdma(k, k + 1); hdma(h0a, 0, 0, k, k + 1)
            wdma(8, 12); hdma(h0a, 0, 0, 8, 12)
            # tiny consts (needed by finalize(0) ~51us; land ~30us from here)
            nc.sync.dma_start(ident[:], identd[:, :])
            nc.sync.dma_start(ones_sb[:], onesd[:, :])
            wdma(12, 16); hdma(h0a, 0, 0, 12, 16)
            h0b = halloc(0, 1)
            wdma(16, 20); hdma(h0b, 0, 1, 0, 4)
            wdma(20, 24); hdma(h0b, 0, 1, 4, 8)
            wdma(24, 28); hdma(h0b, 0, 1, 8, 12)
            wdma(28, 32); hdma(h0b, 0, 1, 12, 16)
            # cos/sin land right behind the w/h0 stream (~49us; needed ~51us)
            nc.sync.dma_start(cos_sb[:], cosT[:, :])
            nc.sync.dma_start(sin_sb[:], sinT[:, :])

            halves = {(0, 0): h0a, (0, 1): h0b}

            def transposes(ch, ps_pool):
                # v transpose via REGULAR matmul against identity: out = vT.T
                for jj in range(4):
                    pst = ps_pool.tile([128, 128], F32, tag="psT")
                    nc.tensor.matmul(pst[:], qk_c[5][ch][:, jj * 128:(jj + 1) * 128],
                                     ident[:], start=True, stop=True)
                    nc.vector.tensor_copy(v_c[ch][:, jj, :], pst[:])

            def rope_m(ch, m, pool):
                # fused RoPE: x = x*cos + shuf(x)*sin_signed; shuf is the
                # rotate-half partition permutation done by DMA (not the
                # tensor engine); the sign lives in the host sin table
                asl = slice(ch * ACH, (ch + 1) * ACH)
                qk = qk_c[m][ch]
                shf = pool.tile([128, ACH], BF, tag="shf")
                nc.sync.dma_start(shf[0:64, :], qk[64:128, :])
                nc.sync.dma_start(shf[64:128, :], qk[0:64, :])
                tmp = pool.tile([128, ACH], BF, tag="tmp")
                nc.vector.tensor_mul(tmp[:], shf[:], sin_sb[:, asl])
                nc.vector.tensor_mul(qk[:], qk[:], cos_sb[:, asl])
                nc.vector.tensor_add(qk[:], qk[:], tmp[:])

            def finalize(ch, full=True):
                ps = chains[ch]
                if full:
                    # drain PSUM on the (idle) scalar engine: it leaves vector
                    # free for RoPE
                    nc.scalar.copy(qk_c[5][ch][:], ps[5][:])
                    for m in range(5):
                        nc.scalar.copy(qk_c[m][ch][:], ps[m][:])
                    transposes(ch, psT)
                    for m in range(5):
                        rope_m(ch, m, tmp_pool)
                else:
                    # last chunk: drains split scalar/vector, ordered to free
                    # the PSUM banks in the order phase B will claim them
                    # (ss j0->bank0=m0, ss j1->bank1=m1, ss j2->bank2=m2,
                    #  pa->bank3=m3, bank4=m4; bank5=m5 feeds psD much later).
                    nc.scalar.copy(qk_c[0][ch][:], ps[0][:])
                    nc.vector.tensor_copy(qk_c[1][ch][:], ps[1][:])
                    nc.scalar.copy(qk_c[3][ch][:], ps[3][:])
                    nc.vector.tensor_copy(qk_c[2][ch][:], ps[2][:])
                    nc.scalar.copy(qk_c[4][ch][:], ps[4][:])
                    nc.scalar.copy(qk_c[5][ch][:], ps[5][:])
                    # transposes + RoPE for this chunk are emitted interleaved
                    # into the first phase-B block (nothing there needs them)

            chains = {}
            for ch in range(NCHA):
                ha = halves.pop((ch, 0))
                hb = halves.pop((ch, 1))
                ps = [psA.tile([128, ACH], F32, tag="psA", name=f"ps{ch}_{m}")
                      for m in range(6)]
                chains[ch] = ps
                for k in range(KO):
                    src = ha if k < 16 else hb
                    kl = k % 16
                    # on the very last k-step, stop the chains in the order the
                    # drains will run so the drains pipeline into the A tail
                    morder = [0, 3, 1, 2, 4, 5] if (ch == 3 and k == KO - 1) \
                        else range(6)
                    for m in morder:
                        nc.tensor.matmul(
                            ps[m][:],
                            w_sb[:, k * MW + m * 128:k * MW + (m + 1) * 128],
                            src[:, kl * ACH:(kl + 1) * ACH],
                            start=(k == 0), stop=(k == KO - 1))
                    # paced DMA emission for upcoming data
                    if ch == 0:
                        if k == 2:
                            nh = halloc(1, 0); halves[(1, 0)] = nh
                            hdma(nh, 1, 0, 0, 4)
                        elif k == 6:
                            hdma(halves[(1, 0)], 1, 0, 4, 16)
                        elif k == 18:
                            nh = halloc(1, 1); halves[(1, 1)] = nh
                            hdma(nh, 1, 1, 0, 4)
                            hdma(nh, 1, 1, 4, 16)
                        elif k == 26:
                            nc.sync.dma_start(mask_sb[:], masksp[:, :])
                    elif ch < 3:
                        if k == 2:
                            nh = halloc(ch + 1, 0); halves[(ch + 1, 0)] = nh
                            hdma(nh, ch + 1, 0, 0, 4)
                            hdma(nh, ch + 1, 0, 4, 16)
                        elif k == 6 and ch == 1:
                            nc.sync.dma_start(wo_sb[:, :HPC * MO * 64],
                                              wop[:, :HPC * MO * 64])
                        elif k == 14 and ch == 1:
                            nc.sync.dma_start(wo_sb[:, HPC * MO * 64:],
                                              wop[:, HPC * MO * 64:])
                        elif k == 18:
                            nh = halloc(ch + 1, 1); halves[(ch + 1, 1)] = nh
                            hdma(nh, ch + 1, 1, 0, 4)
                            hdma(nh, ch + 1, 1, 4, 16)
                finalize(ch, full=(ch < 3))

        # ---------------- Phase B + C pools ----------------
        tmp_pool2 = ctx.enter_context(tc.tile_pool(name="ropetmp2", bufs=2))
        probs_pool = ctx.enter_context(tc.tile_pool(name="probs", bufs=11))
        gsum_pool = ctx.enter_context(tc.tile_pool(name="gsum", bufs=8))
        attn_pool = ctx.enter_context(tc.tile_pool(name="attnT", bufs=1))
        attn_c = [[attn_pool.tile([128, CH], BF, name=f"attn{h}_{ci}")
                   for ci in range(NCHA)] for h in range(HPC)]
        rden_pool = ctx.enter_context(tc.tile_pool(name="rden", bufs=2))
        out_pool = ctx.enter_context(tc.tile_pool(name="outstage", bufs=2))
        psS = ctx.enter_context(tc.tile_pool(name="psS", bufs=3, space="PSUM"))
        psAcc = ctx.enter_context(tc.tile_pool(name="psAcc", bufs=2, space="PSUM"))
        psD = ctx.enter_context(tc.tile_pool(name="psD", bufs=1, space="PSUM"))

        def phase_c(ci, groups, fine=False):
            # out[:, mo-group, chunk] = Wo_c.T @ attn  (8 mo per staged DMA;
            # fine=True DMAs per 2 mo so the final drain overlaps compute)
            w = CH
            sl = slice(ci * CH, (ci + 1) * CH)
            for g in groups:
                ob = out_pool.tile([128, 8, CH], BF, tag="ob")
                for gi in range(8):
                    mo = g * 8 + gi
                    po = psO.tile([128, CH], F32, tag="po")
                    for kk in range(HPC):
                        woff = (kk * MO + mo) * 128
                        nc.tensor.matmul(po[:, :w], wo_sb[:, woff:woff + 128],
                                         attn_c[kk][ci][:, :w],
                                         start=(kk == 0), stop=(kk == HPC - 1))
                    if gi % 2 == 0:
                        nc.scalar.copy(ob[:, gi, :w], po[:, :w])
                    else:
                        nc.vector.tensor_copy(ob[:, gi, :w], po[:, :w])
                    if fine and gi % 2 == 1:
                        nc.sync.dma_start(
                            outT[:, g * 8 + gi - 1:g * 8 + gi + 1, sl],
                            ob[:, gi - 1:gi + 1, :w])
                if not fine:
                    nc.sync.dma_start(outT[:, g * 8:(g + 1) * 8, sl],
                                      ob[:, :, :w])

        def phase_b(ci, h, use_par=True):
            tq0 = ci * CH
            w = CH
            jb = tq0 // 128
            nts = jb + w // 128
            pa = psAcc.tile([128, CH], F32, tag="pa")
            if not use_par:
                pd = psD.tile([128, CH], F32, tag="pd")

            # deferred-by-one pipeline: emit ss/exp for j, then math for j-1
            pend = [None]  # (j, pr, off)
            grp = []       # prob tiles awaiting group sum (ALL tiles join:
            # diag tiles are zero-filled left of `off` by a gpsimd memset, so
            # one group-sum covers 8 tiles, diag included)
            gs = []        # per-group tree-sum results

            def flush_prev(last):
                if pend[0] is None:
                    return
                j, pr, off = pend[0]
                pend[0] = None
                nc.tensor.matmul(pa[:, off:w], v_c[j // 4][:, j % 4, :],
                                 pr[:, off:w],
                                 start=(j == 0), stop=(j == nts - 1))
                grp.append(pr)
                if len(grp) == 8 or last:
                    # pairwise tree-sum on DVE
                    lvl = list(grp)
                    while len(lvl) > 1:
                        nxt = []
                        for a in range(0, len(lvl) - 1, 2):
                            s = gsum_pool.tile([128, CH], BF, tag="g")
                            nc.vector.tensor_add(s[:, :w], lvl[a][:, :w],
                                                 lvl[a + 1][:, :w])
                            nxt.append(s)
                        if len(lvl) % 2:
                            nxt.append(lvl[-1])
                        lvl = nxt
                    gs.append(lvl[0])
                    grp.clear()

            for j in range(nts):
                i = j - jb
                off = 0 if i < 1 else min(i * 128, w - 128)
                ss = psS.tile([128, CH], F32, tag="psS")
                nc.tensor.matmul(ss[:, off:w],
                                 qk_c[4][j // 4][:, (j % 4) * 128:(j % 4 + 1) * 128],
                                 qk_c[h][ci][:, off:w],
                                 start=True, stop=True)
                pr = probs_pool.tile([128, CH], BF, tag="probs")
                if off > 0:
                    # zero the left region on the idle gpsimd engine so this
                    # (masked) diag tile can join a denominator group-sum
                    nc.gpsimd.memset(pr[:, :off], 0.0)
                nc.scalar.activation(pr[:, off:w], ss[:, off:w], AF.Exp,
                                     scale=SCALING)
                if i >= 0:
                    nc.vector.tensor_mul(pr[:, off:w], pr[:, off:w],
                                         mask_sb[:, i * CH + off:i * CH + w])
                flush_prev(last=False)
                pend[0] = (j, pr, off)

            def fin():
                # deferred epilogue: emitted after the next interleaved
                # phase-C group so the tensor queue never HOL-blocks on the
                # vector tree-sum here
                flush_prev(last=True)
                while len(gs) > 1:
                    s = gsum_pool.tile([128, CH], BF, tag="g")
                    nc.vector.tensor_add(s[:, :w], gs[0][:, :w], gs[1][:, :w])
                    gs[:2] = [s]
                rden = rden_pool.tile([128, CH], F32, tag="rden")
                if use_par:
                    # cross-partition sum on the (idle) gpsimd engine instead
                    # of a ones-matmul on the PE
                    pdsb = rden_pool.tile([128, CH], F32, tag="pdsb")
                    nc.gpsimd.partition_all_reduce(
                        pdsb[:, :w], gs[0][:, :w], channels=128,
                        reduce_op=bass_isa.ReduceOp.add)
                    nc.vector.reciprocal_approx_fast(out=rden[:, :w],
                                                     in_=pdsb[:, :w])
                else:
                    nc.tensor.matmul(pd[:, :w], ones_sb[:], gs[0][:, :w],
                                     start=True, stop=True)
                    nc.vector.reciprocal_approx_fast(out=rden[:, :w],
                                                     in_=pd[:, :w])
                nc.vector.tensor_mul(attn_c[h][ci][:, :w], pa[:, :w],
                                     rden[:, :w])
            return fin

        # Chunk order: mid/deep 512-chunks first (dense tensor work densifies
        # the A->B transition); the shallow chunk 0 lands at the end, its exps
        # hidden under interleaved C. ch3's transposes + RoPE (deferred from
        # finalize) interleave into the first B chunk: nothing there needs
        # them, and their engine work hides behind B's execution.
        with tc.tile_pool(name="psX", bufs=1, space="PSUM") as psX:
            fin = phase_b(1, 0)
            transposes(3, psX)
            fin()
            fin = phase_b(1, 1)
            rope_m(3, 0, tmp_pool2)
            rope_m(3, 1, tmp_pool2)
            fin()
            fin = phase_b(1, 2)
            rope_m(3, 2, tmp_pool2)
            rope_m(3, 3, tmp_pool2)
            fin()
            fin = phase_b(1, 3)
            rope_m(3, 4, tmp_pool2)
            fin()

        psO = ctx.enter_context(tc.tile_pool(name="psO", bufs=2, space="PSUM"))
        ORDER = [1, 2, 3, 0]
        for idx in range(1, len(ORDER)):
            ci = ORDER[idx]
            for h in range(HPC):
                fin = phase_b(ci, h, use_par=(ci != 0))
                phase_c(ORDER[idx - 1], [h])
                fin()
        phase_c(ORDER[-1], [0, 1, 2, 3], fine=True)

    nc.compile()
    return nc


def get_nc():
    if "nc" not in _CACHE:
        _CACHE["nc"] = _build_nc()
    return _CACHE["nc"]


def _bf(x):
    return np.ascontiguousarray(np.asarray(x, dtype=np.float32).astype(BF16NP))


def prep_in_maps(hidden_states, cos, sin, Wq, Wk, Wv, Wo):
    # hidp[p, (ch*KO + ko)*ACH + t] = hidden[ch*ACH + t, ko*128 + p]
    hidT = np.asarray(hidden_states, dtype=np.float32).T  # [HID, T]
    hidp = _bf(hidT.reshape(KO, 128, NCHA, ACH).transpose(1, 2, 0, 3)
               .reshape(128, -1))
    cosT = _bf(np.asarray(cos).T)
    sinT_ = np.asarray(sin, dtype=np.float32).T.copy()
    sinT_[:D // 2, :] *= -1.0  # sign of rotate-half folded into sin table
    sinT = _bf(sinT_)
    # masksp[p, i*CH+f] = 1 if i*128+p <= f
    i_idx = np.arange(4)[None, :, None] * 128
    p_idx = np.arange(128)[:, None, None]
    f_idx = np.arange(CH)[None, None, :]
    masksp = _bf(((i_idx + p_idx) <= f_idx).astype(np.float32).reshape(128, -1))

    Wq = np.asarray(Wq, dtype=np.float32)
    Wk = np.asarray(Wk, dtype=np.float32)
    Wv = np.asarray(Wv, dtype=np.float32)
    Wo = np.asarray(Wo, dtype=np.float32)

    in_maps = []
    for c in range(NCORES):
        wqkv = np.concatenate([
            Wq[:, c * QW:(c + 1) * QW],
            Wk[:, c * D:(c + 1) * D],
            Wv[:, c * D:(c + 1) * D],
        ], axis=1)  # [HID, MW]
        # wqkvp[p, ko*MW+m] = wqkv[ko*128+p, m]
        wqkvp = _bf(wqkv.reshape(KO, 128, MW).transpose(1, 0, 2).reshape(128, -1))
        # wop[p, (kk*MO+mo)*128+q] = Wo[c*QW + kk*128+p, mo*128+q]
        wo_c = Wo[c * QW:(c + 1) * QW, :]
        wop_ = _bf(wo_c.reshape(HPC, 128, MO, 128).transpose(1, 0, 2, 3)
                   .reshape(128, -1))
        in_maps.append({
            "hidp": hidp,
            "wqkvp": wqkvp,
            "wop": wop_,
            "cosT": cosT,
            "sinT": sinT,
            "onesd": np.ones((128, 128), dtype=BF16NP),
            "identd": np.eye(128, dtype=np.float32).astype(BF16NP),
            "masksp": masksp,
        })
    return in_maps


def postprocess(results):
    # outT_p: [128, MO, T] bf16, out[t, mo*128+p] = outT_p[p, mo, t]
    acc = None
    for r in results:
        part = r["outT_p"].astype(np.float32)
        acc = part if acc is None else acc + part
    out = acc.transpose(1, 0, 2).reshape(HID, T)  # [HID, T]
    return np.ascontiguousarray(out.T).astype(np.float32)


def kernel(hidden_states, position_ids, cos, sin, Wq, Wk, Wv, Wo):
    from concourse.bass_utils import run_bass_kernel_spmd
    nc = get_nc()
    in_maps = prep_in_maps(hidden_states, cos, sin, Wq, Wk, Wv, Wo)
    res = run_bass_kernel_spmd(nc, in_maps, core_ids=list(range(NCORES)))
    return postprocess(res.results)


# revision 11
# speedup vs baseline: 1.1087x; 1.1087x over previous
"""Trainium2 Bass kernel for nn_LlamaAttention (T=2048, HID=4096, HQ=32, HKV=8, D=128).

Tensor-parallel over heads across 8 NeuronCores: core c owns q-heads 4c..4c+3 and
kv-head c (GQA group size 4 == heads-per-core, so attention is fully core-local).
Wo is row-sharded; each core computes a partial [HID, T] output (transposed, bf16)
and the host sums the 8 partials. No device collectives.

v4 vs v2:
 - per-chunk tiles for q/kT/vT/v/attn: kills the tile-granular false deps that
   made the first B matmuls wait on the last A-chunk PSUM drains (incl. the HAM
   re-throttle the bubble caused)
 - last A chunk: k=31 emits chains in m-order [0,3,1,2,4,5] and the drains split
   scalar/vector ordered to match B's PSUM-bank reuse order, so B's first
   score/AV matmuls find their banks free
 - v transposes as REGULAR matmuls vs identity (not transpose-mode): ~81ns vs
   ~311ns each, and they count as PE-busy for HAM
 - denominator: one ones-matmul per (head, chunk) -- group tree-sums combined
   on DVE first (was: one per 8-tile group)
 - final phase-C output DMAs at 2-mo granularity so the last transfer after the
   last matmul is 256KB, not 512KB+queue
 (tried and reverted: HAM warm-up via dummy matmuls -- phase A's first chunk is
  DMA-delivery-bound so the cold start was free pacing, and the warm PE just
  starved + re-throttled; gpsimd partition_all_reduce for the denominator --
  3.6us per call and it poisons the pa-bank recycle path)

Self-contained: hardcodes all shapes; builds the Bass kernel once per process.
"""
import numpy as np
import ml_dtypes

T, HID, HQ, HKV, D = 2048, 4096, 32, 8, 128
NCORES = 8
HPC = HQ // NCORES            # 4 q heads per core
QW = HPC * D                  # 512 q columns per core
MW = QW + 2 * D               # 768 qkv columns per core
KO = HID // 128               # 32 k-tiles
ACH = 512                     # phase A t-chunk width (PSUM bank)
NCHA = T // ACH               # 4
CH = 512                      # attention tq chunk width (PSUM bank)
MO = HID // 128               # 32 output row-tiles
SCALING = float(D) ** -0.5
BF16NP = ml_dtypes.bfloat16

_CACHE = {}


def _build_nc():
    import concourse.mybir as mybir
    import concourse.tile as tile
    from concourse import bacc
    from contextlib import ExitStack

    F32 = mybir.dt.float32
    BF = mybir.dt.bfloat16
    AF = mybir.ActivationFunctionType

    nc = bacc.Bacc("TRN2", target_bir_lowering=False, debug=False,
                   dynamic_dma_scratch_size=2048)

    # pre-swizzled inputs (see prep_in_maps)
    hidp = nc.dram_tensor("hidp", [128, NCHA * KO * ACH], BF, kind="ExternalInput")
    wqkvp = nc.dram_tensor("wqkvp", [128, KO * MW], BF, kind="ExternalInput")
    wop = nc.dram_tensor("wop", [128, HPC * MO * 128], BF, kind="ExternalInput")
    cosT = nc.dram_tensor("cosT", [D, T], BF, kind="ExternalInput")
    sinT = nc.dram_tensor("sinT", [D, T], BF, kind="ExternalInput")
    onesd = nc.dram_tensor("onesd", [128, 128], BF, kind="ExternalInput")
    identd = nc.dram_tensor("identd", [128, 128], BF, kind="ExternalInput")
    masksp = nc.dram_tensor("masksp", [128, 4 * CH], BF, kind="ExternalInput")
    outT = nc.dram_tensor("outT_p", [128, MO, T], BF, kind="ExternalOutput")

    with tile.TileContext(nc) as tc, ExitStack() as ctx:
        consts = ctx.enter_context(tc.tile_pool(name="consts", bufs=1))
        ones_sb = consts.tile([128, 128], BF)
        ident = consts.tile([128, 128], BF)
        mask_sb = consts.tile([128, 4 * CH], BF)
        cos_sb = consts.tile([128, T], BF)
        sin_sb = consts.tile([128, T], BF)

        qkv_pool = ctx.enter_context(tc.tile_pool(name="qkvT", bufs=1))
        # qk_c[m][ch]: m=0..3 q heads, m=4 kT, m=5 vT -- per-chunk [128, 512]
        qk_c = [[qkv_pool.tile([128, ACH], BF, name=f"qk{m}_{ch}")
                 for ch in range(NCHA)] for m in range(6)]
        # v_c[ch][s, jj, d]: per-chunk transposed V (4 j-tiles per chunk)
        v_c = [qkv_pool.tile([128, 4, 128], BF, name=f"v_{ch}")
               for ch in range(NCHA)]

        wopool = ctx.enter_context(tc.tile_pool(name="wo", bufs=1))
        wo_sb = wopool.tile([128, HPC * MO * 128], BF)

        # ---- Phase A: qkvT = wqkv.T @ hidT, one 32-step PSUM chain per (ch,m) --
        with tc.tile_pool(name="w", bufs=1) as wp, \
             tc.tile_pool(name="hid", bufs=3) as hp, \
             tc.tile_pool(name="ropetmp", bufs=2) as tmp_pool, \
             tc.tile_pool(name="psA", bufs=6, space="PSUM") as psA, \
             tc.tile_pool(name="psT", bufs=2, space="PSUM") as psT:
            w_sb = wp.tile([128, KO * MW], BF, name="w_sb")

            def wdma(k0, k1):
                nc.sync.dma_start(w_sb[:, k0 * MW:k1 * MW], wqkvp[:, k0 * MW:k1 * MW])

            HHALF = 16 * ACH  # half-chunk: 16 k-tiles

            def hdma(ht, ch, half, kk0, kk1):
                # local k-tile range [kk0,kk1) of half `half` of chunk ch
                base = (ch * KO + half * 16) * ACH
                nc.sync.dma_start(ht[:, kk0 * ACH:kk1 * ACH],
                                  hidp[:, base + kk0 * ACH:base + kk1 * ACH])

            def halloc(ch, half):
                return hp.tile([128, HHALF], BF, tag="hid", name=f"h{ch}_{half}")

            # startup: k0-2 of w AND hid land before the chain's first matmul
            # (which waits on w0+h0): a ~2-3 k-tile buffer absorbs DMA jitter
            # on a stream that otherwise runs at ~91% of delivery bandwidth.
            h0a = halloc(0, 0)
            wdma(0, 1); hdma(h0a, 0, 0, 0, 1)
            wdma(1, 3); hdma(h0a, 0, 0, 1, 3)
            for k in range(3, 8):
                wdma(k, k + 1); hdma(h0a, 0, 0, k, k + 1)
            wdma(8, 12); hdma(h0a, 0, 0, 8, 12)
            # tiny consts (needed by finalize(0) ~51us; land ~30us from here)
            nc.sync.dma_start(ident[:], identd[:, :])
            nc.sync.dma_start(ones_sb[:], onesd[:, :])
            wdma(12, 16); hdma(h0a, 0, 0, 12, 16)
            h0b = halloc(0, 1)
            wdma(16, 20); hdma(h0b, 0, 1, 0, 4)
            wdma(20, 24); hdma(h0b, 0, 1, 4, 8)
            wdma(24, 28); hdma(h0b, 0, 1, 8, 12)
            wdma(28, 32); hdma(h0b, 0, 1, 12, 16)
            # cos/sin land right behind the w/h0 stream (~49us; needed ~51us)
            nc.sync.dma_start(cos_sb[:], cosT[:, :])
            nc.sync.dma_start(sin_sb[:], sinT[:, :])

            halves = {(0, 0): h0a, (0, 1): h0b}

            def transposes(ch, ps_pool):
                # v transpose via REGULAR matmul against identity: out = vT.T
                for jj in range(4):
                    pst = ps_pool.tile([128, 128], F32, tag="psT")
                    nc.tensor.matmul(pst[:], qk_c[5][ch][:, jj * 128:(jj + 1) * 128],
                                     ident[:], start=True, stop=True)
                    nc.vector.tensor_copy(v_c[ch][:, jj, :], pst[:])

            def rope_m(ch, m, pool):
                # fused RoPE: x = x*cos + shuf(x)*sin_signed; shuf is the
                # rotate-half partition permutation done by DMA (not the
                # tensor engine); the sign lives in the host sin table
                asl = slice(ch * ACH, (ch + 1) * ACH)
                qk = qk_c[m][ch]
                shf = pool.tile([128, ACH], BF, tag="shf")
                nc.sync.dma_start(shf[0:64, :], qk[64:128, :])
                nc.sync.dma_start(shf[64:128, :], qk[0:64, :])
                tmp = pool.tile([128, ACH], BF, tag="tmp")
                nc.vector.tensor_mul(tmp[:], shf[:], sin_sb[:, asl])
                nc.vector.tensor_mul(qk[:], qk[:], cos_sb[:, asl])
                nc.vector.tensor_add(qk[:], qk[:], tmp[:])

            def finalize(ch, full=True):
                ps = chains[ch]
                if full:
                    # drain PSUM on the (idle) scalar engine: it leaves vector
                    # free for RoPE
                    nc.scalar.copy(qk_c[5][ch][:], ps[5][:])
                    for m in range(5):
                        nc.scalar.copy(qk_c[m][ch][:], ps[m][:])
                    transposes(ch, psT)
                    for m in range(5):
                        rope_m(ch, m, tmp_pool)
                else:
                    # last chunk: drains split scalar/vector, ordered to free
                    # the PSUM banks in the order phase B will claim them
                    # (ss j0->bank0=m0, ss j1->bank1=m1, ss j2->bank2=m2,
                    #  pa->bank3=m3, bank4=m4; bank5=m5 feeds psD much later).
                    nc.scalar.copy(qk_c[0][ch][:], ps[0][:])
                    nc.vector.tensor_copy(qk_c[1][ch][:], ps[1][:])
                    nc.scalar.copy(qk_c[3][ch][:], ps[3][:])
                    nc.vector.tensor_copy(qk_c[2][ch][:], ps[2][:])
                    nc.scalar.copy(qk_c[4][ch][:], ps[4][:])
                    nc.scalar.copy(qk_c[5][ch][:], ps[5][:])
                    # transposes + RoPE for this chunk are emitted interleaved
                    # into the first phase-B block (nothing there needs them)

            chains = {}
            for ch in range(NCHA):
                ha = halves.pop((ch, 0))
                hb = halves.pop((ch, 1))
                ps = [psA.tile([128, ACH], F32, tag="psA", name=f"ps{ch}_{m}")
                      for m in range(6)]
                chains[ch] = ps
                for k in range(KO):
                    src = ha if k < 16 else hb
                    kl = k % 16
                    # on the very last k-step, stop the chains in the order the
                    # drains will run so the drains pipeline into the A tail
                    morder = [0, 3, 1, 2, 4, 5] if (ch == 3 and k == KO - 1) \
                        else range(6)
                    for m in morder:
                        nc.tensor.matmul(
                            ps[m][:],
                            w_sb[:, k * MW + m * 128:k * MW + (m + 1) * 128],
                            src[:, kl * ACH:(kl + 1) * ACH],
                            start=(k == 0), stop=(k == KO - 1))
                    # paced DMA emission for upcoming data
                    if ch == 0:
                        if k == 2:
                            nh = halloc(1, 0); halves[(1, 0)] = nh
                            hdma(nh, 1, 0, 0, 4)
                        elif k == 6:
                            hdma(halves[(1, 0)], 1, 0, 4, 16)
                        elif k == 18:
                            nh = halloc(1, 1); halves[(1, 1)] = nh
                            hdma(nh, 1, 1, 0, 4)
                            hdma(nh, 1, 1, 4, 16)
                        elif k == 26:
                            nc.sync.dma_start(mask_sb[:], masksp[:, :])
                    elif ch < 3:
                        if k == 2:
                            nh = halloc(ch + 1, 0); halves[(ch + 1, 0)] = nh
                            hdma(nh, ch + 1, 0, 0, 4)
                            hdma(nh, ch + 1, 0, 4, 16)
                        elif k == 6 and ch == 1:
                            nc.sync.dma_start(wo_sb[:, :HPC * MO * 64],
                                              wop[:, :HPC * MO * 64])
                        elif k == 14 and ch == 1:
                            nc.sync.dma_start(wo_sb[:, HPC * MO * 64:],
                                              wop[:, HPC * MO * 64:])
                        elif k == 18:
                            nh = halloc(ch + 1, 1); halves[(ch + 1, 1)] = nh
                            hdma(nh, ch + 1, 1, 0, 4)
                            hdma(nh, ch + 1, 1, 4, 16)
                finalize(ch, full=(ch < 3))

        # ---------------- Phase B + C pools ----------------
        tmp_pool2 = ctx.enter_context(tc.tile_pool(name="ropetmp2", bufs=2))
        probs_pool = ctx.enter_context(tc.tile_pool(name="probs", bufs=11))
        gsum_pool = ctx.enter_context(tc.tile_pool(name="gsum", bufs=8))
        attn_pool = ctx.enter_context(tc.tile_pool(name="attnT", bufs=1))
        attn_c = [[attn_pool.tile([128, CH], BF, name=f"attn{h}_{ci}")
                   for ci in range(NCHA)] for h in range(HPC)]
        rden_pool = ctx.enter_context(tc.tile_pool(name="rden", bufs=2))
        out_pool = ctx.enter_context(tc.tile_pool(name="outstage", bufs=2))
        psS = ctx.enter_context(tc.tile_pool(name="psS", bufs=3, space="PSUM"))
        psAcc = ctx.enter_context(tc.tile_pool(name="psAcc", bufs=2, space="PSUM"))
        psD = ctx.enter_context(tc.tile_pool(name="psD", bufs=1, space="PSUM"))

        def phase_c(ci, groups, fine=False):
            # out[:, mo-group, chunk] = Wo_c.T @ attn  (8 mo per staged DMA;
            # fine=True DMAs per 2 mo so the final drain overlaps compute)
            w = CH
            sl = slice(ci * CH, (ci + 1) * CH)
            for g in groups:
                ob = out_pool.tile([128, 8, CH], BF, tag="ob")
                for gi in range(8):
                    mo = g * 8 + gi
                    po = psO.tile([128, CH], F32, tag="po")
                    for kk in range(HPC):
                        woff = (kk * MO + mo) * 128
                        nc.tensor.matmul(po[:, :w], wo_sb[:, woff:woff + 128],
                                         attn_c[kk][ci][:, :w],
                                         start=(kk == 0), stop=(kk == HPC - 1))
                    if gi % 2 == 0:
                        nc.scalar.copy(ob[:, gi, :w], po[:, :w])
                    else:
                        nc.vector.tensor_copy(ob[:, gi, :w], po[:, :w])
                    if fine and gi % 2 == 1:
                        nc.sync.dma_start(
                            outT[:, g * 8 + gi - 1:g * 8 + gi + 1, sl],
                            ob[:, gi - 1:gi + 1, :w])
                if not fine:
                    nc.sync.dma_start(outT[:, g * 8:(g + 1) * 8, sl],
                                      ob[:, :, :w])

        def phase_b(ci, h):
            tq0 = ci * CH
            w = CH
            jb = tq0 // 128
            nts = jb + w // 128
            pa = psAcc.tile([128, CH], F32, tag="pa")
            pd = psD.tile([128, CH], F32, tag="pd")

            # deferred-by-one pipeline: emit ss/exp for j, then math for j-1
            pend = [None]  # (j, pr, off)
            grp = []       # prob tiles awaiting group sum (ALL tiles join:
            # diag tiles are zero-filled left of `off` by a gpsimd memset, so
            # one group-sum covers 8 tiles, diag included)
            gs = []        # per-group tree-sum results

            def flush_prev(last):
                if pend[0] is None:
                    return
                j, pr, off = pend[0]
                pend[0] = None
                nc.tensor.matmul(pa[:, off:w], v_c[j // 4][:, j % 4, :],
                                 pr[:, off:w],
                                 start=(j == 0), stop=(j == nts - 1))
                grp.append(pr)
                if len(grp) == 8 or last:
                    # pairwise tree-sum on DVE
                    lvl = list(grp)
                    while len(lvl) > 1:
                        nxt = []
                        for a in range(0, len(lvl) - 1, 2):
                            s = gsum_pool.tile([128, CH], BF, tag="g")
                            nc.vector.tensor_add(s[:, :w], lvl[a][:, :w],
                                                 lvl[a + 1][:, :w])
                            nxt.append(s)
                        if len(lvl) % 2:
                            nxt.append(lvl[-1])
                        lvl = nxt
                    gs.append(lvl[0])
                    grp.clear()

            for j in range(nts):
                i = j - jb
                off = 0 if i < 1 else min(i * 128, w - 128)
                ss = psS.tile([128, CH], F32, tag="psS")
                nc.tensor.matmul(ss[:, off:w],
                                 qk_c[4][j // 4][:, (j % 4) * 128:(j % 4 + 1) * 128],
                                 qk_c[h][ci][:, off:w],
                                 start=True, stop=True)
                pr = probs_pool.tile([128, CH], BF, tag="probs")
                if off > 0:
                    # zero the left region on the idle gpsimd engine so this
                    # (masked) diag tile can join a denominator group-sum
                    nc.gpsimd.memset(pr[:, :off], 0.0)
                nc.scalar.activation(pr[:, off:w], ss[:, off:w], AF.Exp,
                                     scale=SCALING)
                if i >= 0:
                    nc.vector.tensor_mul(pr[:, off:w], pr[:, off:w],
                                         mask_sb[:, i * CH + off:i * CH + w])
                flush_prev(last=False)
                pend[0] = (j, pr, off)

            def fin():
                # deferred epilogue: emitted after the next interleaved
                # phase-C group so the tensor queue never HOL-blocks on the
                # vector tree-sum here
                flush_prev(last=True)
                while len(gs) > 1:
                    s = gsum_pool.tile([128, CH], BF, tag="g")
                    nc.vector.tensor_add(s[:, :w], gs[0][:, :w], gs[1][:, :w])
                    gs[:2] = [s]
                rden = rden_pool.tile([128, CH], F32, tag="rden")
                nc.tensor.matmul(pd[:, :w], ones_sb[:], gs[0][:, :w],
                                 start=True, stop=True)
                nc.vector.reciprocal_approx_fast(out=rden[:, :w],
                                                 in_=pd[:, :w])
                nc.vector.tensor_mul(attn_c[h][ci][:, :w], pa[:, :w],
                                     rden[:, :w])
            return fin

        # Chunk order: mid/deep 512-chunks first (dense tensor work densifies
        # the A->B transition); the shallow chunk 0 lands at the end, its exps
        # hidden under interleaved C. ch3's transposes + RoPE (deferred from
        # finalize) interleave into the first B chunk: nothing there needs
        # them, and their engine work hides behind B's execution.
        with tc.tile_pool(name="psX", bufs=1, space="PSUM") as psX:
            fin = phase_b(1, 0)
            transposes(3, psX)
            fin()
            fin = phase_b(1, 1)
            rope_m(3, 0, tmp_pool2)
            rope_m(3, 1, tmp_pool2)
            fin()
            fin = phase_b(1, 2)
            rope_m(3, 2, tmp_pool2)
            rope_m(3, 3, tmp_pool2)
            fin()
            fin = phase_b(1, 3)
            rope_m(3, 4, tmp_pool2)
            fin()

        psO = ctx.enter_context(tc.tile_pool(name="psO", bufs=2, space="PSUM"))
        ORDER = [1, 2, 3, 0]
        for idx in range(1, len(ORDER)):
            ci = ORDER[idx]
            for h in range(HPC):
                fin = phase_b(ci, h)
                phase_c(ORDER[idx - 1], [h])
                fin()
        phase_c(ORDER[-1], [0, 1, 2, 3], fine=True)

    nc.compile()
    return nc


def get_nc():
    if "nc" not in _CACHE:
        _CACHE["nc"] = _build_nc()
    return _CACHE["nc"]


def _bf(x):
    return np.ascontiguousarray(np.asarray(x, dtype=np.float32).astype(BF16NP))


def prep_in_maps(hidden_states, cos, sin, Wq, Wk, Wv, Wo):
    # hidp[p, (ch*KO + ko)*ACH + t] = hidden[ch*ACH + t, ko*128 + p]
    hidT = np.asarray(hidden_states, dtype=np.float32).T  # [HID, T]
    hidp = _bf(hidT.reshape(KO, 128, NCHA, ACH).transpose(1, 2, 0, 3)
               .reshape(128, -1))
    cosT = _bf(np.asarray(cos).T)
    sinT_ = np.asarray(sin, dtype=np.float32).T.copy()
    sinT_[:D // 2, :] *= -1.0  # sign of rotate-half folded into sin table
    sinT = _bf(sinT_)
    # masksp[p, i*CH+f] = 1 if i*128+p <= f
    i_idx = np.arange(4)[None, :, None] * 128
    p_idx = np.arange(128)[:, None, None]
    f_idx = np.arange(CH)[None, None, :]
    masksp = _bf(((i_idx + p_idx) <= f_idx).astype(np.float32).reshape(128, -1))

    Wq = np.asarray(Wq, dtype=np.float32)
    Wk = np.asarray(Wk, dtype=np.float32)
    Wv = np.asarray(Wv, dtype=np.float32)
    Wo = np.asarray(Wo, dtype=np.float32)

    in_maps = []
    for c in range(NCORES):
        wqkv = np.concatenate([
            Wq[:, c * QW:(c + 1) * QW],
            Wk[:, c * D:(c + 1) * D],
            Wv[:, c * D:(c + 1) * D],
        ], axis=1)  # [HID, MW]
        # wqkvp[p, ko*MW+m] = wqkv[ko*128+p, m]
        wqkvp = _bf(wqkv.reshape(KO, 128, MW).transpose(1, 0, 2).reshape(128, -1))
        # wop[p, (kk*MO+mo)*128+q] = Wo[c*QW + kk*128+p, mo*128+q]
        wo_c = Wo[c * QW:(c + 1) * QW, :]
        wop_ = _bf(wo_c.reshape(HPC, 128, MO, 128).transpose(1, 0, 2, 3)
                   .reshape(128, -1))
        in_maps.append({
            "hidp": hidp,
            "wqkvp": wqkvp,
            "wop": wop_,
            "cosT": cosT,
            "sinT": sinT,
            "onesd": np.ones((128, 128), dtype=BF16NP),
            "identd": np.eye(128, dtype=np.float32).astype(BF16NP),
            "masksp": masksp,
        })
    return in_maps


def postprocess(results):
    # outT_p: [128, MO, T] bf16, out[t, mo*128+p] = outT_p[p, mo, t]
    acc = None
    for r in results:
        part = r["outT_p"].astype(np.float32)
        acc = part if acc is None else acc + part
    out = acc.transpose(1, 0, 2).reshape(HID, T)  # [HID, T]
    return np.ascontiguousarray(out.T).astype(np.float32)


def kernel(hidden_states, position_ids, cos, sin, Wq, Wk, Wv, Wo):
    from concourse.bass_utils import run_bass_kernel_spmd
    nc = get_nc()
    in_maps = prep_in_maps(hidden_states, cos, sin, Wq, Wk, Wv, Wo)
    res = run_bass_kernel_spmd(nc, in_maps, core_ids=list(range(NCORES)))
    return postprocess(res.results)


# revision 17
# speedup vs baseline: 1.1104x; 1.0015x over previous
"""Trainium2 Bass kernel for nn_LlamaAttention (T=2048, HID=4096, HQ=32, HKV=8, D=128).

Tensor-parallel over heads across 8 NeuronCores: core c owns q-heads 4c..4c+3 and
kv-head c (GQA group size 4 == heads-per-core, so attention is fully core-local).
Wo is row-sharded; each core computes a partial [HID, T] output (transposed, bf16)
and the host sums the 8 partials. No device collectives.

v4 vs v2:
 - per-chunk tiles for q/kT/vT/v/attn: kills the tile-granular false deps that
   made the first B matmuls wait on the last A-chunk PSUM drains (incl. the HAM
   re-throttle the bubble caused)
 - last A chunk: k=31 emits chains in m-order [0,3,1,2,4,5] and the drains split
   scalar/vector ordered to match B's PSUM-bank reuse order, so B's first
   score/AV matmuls find their banks free
 - v transposes as REGULAR matmuls vs identity (not transpose-mode): ~81ns vs
   ~311ns each, and they count as PE-busy for HAM
 - denominator: one ones-matmul per (head, chunk) -- group tree-sums combined
   on DVE first (was: one per 8-tile group)
 - final phase-C output DMAs at 2-mo granularity so the last transfer after the
   last matmul is 256KB, not 512KB+queue
 (tried and reverted: HAM warm-up via dummy matmuls -- phase A's first chunk is
  DMA-delivery-bound so the cold start was free pacing, and the warm PE just
  starved + re-throttled; gpsimd partition_all_reduce for the denominator --
  3.6us per call and it poisons the pa-bank recycle path)

Self-contained: hardcodes all shapes; builds the Bass kernel once per process.
"""
import numpy as np
import ml_dtypes

T, HID, HQ, HKV, D = 2048, 4096, 32, 8, 128
NCORES = 8
HPC = HQ // NCORES            # 4 q heads per core
QW = HPC * D                  # 512 q columns per core
MW = QW + 2 * D               # 768 qkv columns per core
KO = HID // 128               # 32 k-tiles
ACH = 512                     # phase A t-chunk width (PSUM bank)
NCHA = T // ACH               # 4
CH = 512                      # attention tq chunk width (PSUM bank)
MO = HID // 128               # 32 output row-tiles
SCALING = float(D) ** -0.5
BF16NP = ml_dtypes.bfloat16

_CACHE = {}


def _build_nc():
    import concourse.mybir as mybir
    import concourse.tile as tile
    from concourse import bacc
    from contextlib import ExitStack

    F32 = mybir.dt.float32
    BF = mybir.dt.bfloat16
    AF = mybir.ActivationFunctionType

    nc = bacc.Bacc("TRN2", target_bir_lowering=False, debug=False,
                   dynamic_dma_scratch_size=2048)

    # pre-swizzled inputs (see prep_in_maps)
    hidp = nc.dram_tensor("hidp", [128, NCHA * KO * ACH], BF, kind="ExternalInput")
    wqkvp = nc.dram_tensor("wqkvp", [128, KO * MW], BF, kind="ExternalInput")
    wop = nc.dram_tensor("wop", [128, HPC * MO * 128], BF, kind="ExternalInput")
    cosT = nc.dram_tensor("cosT", [D, T], BF, kind="ExternalInput")
    sinT = nc.dram_tensor("sinT", [D, T], BF, kind="ExternalInput")
    onesd = nc.dram_tensor("onesd", [128, 128], BF, kind="ExternalInput")
    identd = nc.dram_tensor("identd", [128, 128], BF, kind="ExternalInput")
    masksp = nc.dram_tensor("masksp", [128, 4 * CH], BF, kind="ExternalInput")
    outT = nc.dram_tensor("outT_p", [128, MO, T], BF, kind="ExternalOutput")

    with tile.TileContext(nc) as tc, ExitStack() as ctx:
        consts = ctx.enter_context(tc.tile_pool(name="consts", bufs=1))
        ones_sb = consts.tile([128, 128], BF)
        ident = consts.tile([128, 128], BF)
        mask_sb = consts.tile([128, 4 * CH], BF)
        cos_sb = consts.tile([128, T], BF)
        sin_sb = consts.tile([128, T], BF)

        qkv_pool = ctx.enter_context(tc.tile_pool(name="qkvT", bufs=1))
        # qk_c[m][ch]: m=0..3 q heads, m=4 kT, m=5 vT -- per-chunk [128, 512]
        qk_c = [[qkv_pool.tile([128, ACH], BF, name=f"qk{m}_{ch}")
                 for ch in range(NCHA)] for m in range(6)]
        # v_c[ch][s, jj, d]: per-chunk transposed V (4 j-tiles per chunk)
        v_c = [qkv_pool.tile([128, 4, 128], BF, name=f"v_{ch}")
               for ch in range(NCHA)]

        wopool = ctx.enter_context(tc.tile_pool(name="wo", bufs=1))
        wo_sb = wopool.tile([128, HPC * MO * 128], BF)

        # ---- Phase A: qkvT = wqkv.T @ hidT, one 32-step PSUM chain per (ch,m) --
        with tc.tile_pool(name="w", bufs=1) as wp, \
             tc.tile_pool(name="hid", bufs=3) as hp, \
             tc.tile_pool(name="ropetmp", bufs=2) as tmp_pool, \
             tc.tile_pool(name="psA", bufs=6, space="PSUM") as psA, \
             tc.tile_pool(name="psT", bufs=2, space="PSUM") as psT:
            w_sb = wp.tile([128, KO * MW], BF, name="w_sb")

            def wdma(k0, k1):
                nc.sync.dma_start(w_sb[:, k0 * MW:k1 * MW], wqkvp[:, k0 * MW:k1 * MW])

            HHALF = 16 * ACH  # half-chunk: 16 k-tiles

            def hdma(ht, ch, half, kk0, kk1):
                # local k-tile range [kk0,kk1) of half `half` of chunk ch
                base = (ch * KO + half * 16) * ACH
                nc.sync.dma_start(ht[:, kk0 * ACH:kk1 * ACH],
                                  hidp[:, base + kk0 * ACH:base + kk1 * ACH])

            def halloc(ch, half):
                return hp.tile([128, HHALF], BF, tag="hid", name=f"h{ch}_{half}")

            # startup: k0-2 of w AND hid land before the chain's first matmul
            # (which waits on w0+h0): a ~2-3 k-tile buffer absorbs DMA jitter
            # on a stream that otherwise runs at ~91% of delivery bandwidth.
            # w_k paired with h_k throughout chunk 0 so delivery order matches
            # the chain's consumption order (w-front-loading starved h_k>=16
            # once the A->B fixes let the PE run the early chain at full rate)
            h0a = halloc(0, 0)
            wdma(0, 1); hdma(h0a, 0, 0, 0, 1)
            wdma(1, 2); hdma(h0a, 0, 0, 1, 2)
            wdma(2, 4); hdma(h0a, 0, 0, 2, 4)
            wdma(4, 6); hdma(h0a, 0, 0, 4, 6)
            wdma(6, 8); hdma(h0a, 0, 0, 6, 8)
            # tiny consts (needed by finalize(0) ~51us; land ~30us from here)
            nc.sync.dma_start(ident[:], identd[:, :])
            nc.sync.dma_start(ones_sb[:], onesd[:, :])
            wdma(8, 12); hdma(h0a, 0, 0, 8, 12)
            wdma(12, 16); hdma(h0a, 0, 0, 12, 16)
            h0b = halloc(0, 1)
            wdma(16, 20); hdma(h0b, 0, 1, 0, 4)
            wdma(20, 24); hdma(h0b, 0, 1, 4, 8)
            # cos/sin land mid-stream (~40us; needed by finalize(0) ~51us)
            nc.sync.dma_start(cos_sb[:], cosT[:, :])
            nc.sync.dma_start(sin_sb[:], sinT[:, :])
            wdma(24, 28); hdma(h0b, 0, 1, 8, 12)
            wdma(28, 32); hdma(h0b, 0, 1, 12, 16)

            halves = {(0, 0): h0a, (0, 1): h0b}

            def transposes(ch, ps_pool):
                # v transpose via REGULAR matmul against identity: out = vT.T
                for jj in range(4):
                    pst = ps_pool.tile([128, 128], F32, tag="psT")
                    nc.tensor.matmul(pst[:], qk_c[5][ch][:, jj * 128:(jj + 1) * 128],
                                     ident[:], start=True, stop=True)
                    nc.vector.tensor_copy(v_c[ch][:, jj, :], pst[:])

            def rope_m(ch, m, pool):
                # fused RoPE: x = x*cos + shuf(x)*sin_signed; shuf is the
                # rotate-half partition permutation done by DMA (not the
                # tensor engine); the sign lives in the host sin table
                asl = slice(ch * ACH, (ch + 1) * ACH)
                qk = qk_c[m][ch]
                shf = pool.tile([128, ACH], BF, tag="shf")
                nc.sync.dma_start(shf[0:64, :], qk[64:128, :])
                nc.sync.dma_start(shf[64:128, :], qk[0:64, :])
                tmp = pool.tile([128, ACH], BF, tag="tmp")
                nc.vector.tensor_mul(tmp[:], shf[:], sin_sb[:, asl])
                nc.vector.tensor_mul(qk[:], qk[:], cos_sb[:, asl])
                nc.vector.tensor_add(qk[:], qk[:], tmp[:])

            def finalize(ch, full=True):
                ps = chains[ch]
                if full:
                    # drain PSUM on the (idle) scalar engine: it leaves vector
                    # free for RoPE
                    nc.scalar.copy(qk_c[5][ch][:], ps[5][:])
                    for m in range(5):
                        nc.scalar.copy(qk_c[m][ch][:], ps[m][:])
                    transposes(ch, psT)
                    for m in range(5):
                        rope_m(ch, m, tmp_pool)
                else:
                    # last chunk: drains split 2-scalar/4-vector so the first
                    # EXP of phase B sits behind only 2 drains on the scalar
                    # FIFO; the k=31 m-order lets drains start ~1us before the
                    # chain's last matmul. (order tuned vs the measured PSUM
                    # bank mapping: B's first claims must be drained first)
                    nc.scalar.copy(qk_c[5][ch][:], ps[5][:])
                    nc.vector.tensor_copy(qk_c[1][ch][:], ps[1][:])
                    nc.scalar.copy(qk_c[0][ch][:], ps[0][:])
                    nc.vector.tensor_copy(qk_c[2][ch][:], ps[2][:])
                    nc.vector.tensor_copy(qk_c[3][ch][:], ps[3][:])
                    nc.vector.tensor_copy(qk_c[4][ch][:], ps[4][:])
                    # transposes + RoPE for this chunk are emitted interleaved
                    # into the first phase-B block

            chains = {}
            for ch in range(NCHA):
                ha = halves.pop((ch, 0))
                hb = halves.pop((ch, 1))
                ps = [psA.tile([128, ACH], F32, tag="psA", name=f"ps{ch}_{m}")
                      for m in range(6)]
                chains[ch] = ps
                for k in range(KO):
                    src = ha if k < 16 else hb
                    kl = k % 16
                    # on the very last k-step, stop the chains in the order the
                    # drains will run so the drains pipeline into the A tail
                    morder = [5, 1, 0, 2, 3, 4] if (ch == 3 and k == KO - 1) \
                        else range(6)
                    for m in morder:
                        nc.tensor.matmul(
                            ps[m][:],
                            w_sb[:, k * MW + m * 128:k * MW + (m + 1) * 128],
                            src[:, kl * ACH:(kl + 1) * ACH],
                            start=(k == 0), stop=(k == KO - 1))
                    # paced DMA emission for upcoming data
                    if ch == 0:
                        if k == 2:
                            nh = halloc(1, 0); halves[(1, 0)] = nh
                            hdma(nh, 1, 0, 0, 4)
                        elif k == 6:
                            hdma(halves[(1, 0)], 1, 0, 4, 16)
                        elif k == 18:
                            nh = halloc(1, 1); halves[(1, 1)] = nh
                            hdma(nh, 1, 1, 0, 4)
                            hdma(nh, 1, 1, 4, 16)
                        elif k == 26:
                            nc.sync.dma_start(mask_sb[:], masksp[:, :])
                    elif ch < 3:
                        if k == 2:
                            nh = halloc(ch + 1, 0); halves[(ch + 1, 0)] = nh
                            hdma(nh, ch + 1, 0, 0, 4)
                            hdma(nh, ch + 1, 0, 4, 16)
                        elif k == 6 and ch == 1:
                            nc.sync.dma_start(wo_sb[:, :HPC * MO * 64],
                                              wop[:, :HPC * MO * 64])
                        elif k == 14 and ch == 1:
                            nc.sync.dma_start(wo_sb[:, HPC * MO * 64:],
                                              wop[:, HPC * MO * 64:])
                        elif k == 18:
                            nh = halloc(ch + 1, 1); halves[(ch + 1, 1)] = nh
                            hdma(nh, ch + 1, 1, 0, 4)
                            hdma(nh, ch + 1, 1, 4, 16)
                finalize(ch, full=(ch < 3))

        # ---------------- Phase B + C pools ----------------
        tmp_pool2 = ctx.enter_context(tc.tile_pool(name="ropetmp2", bufs=2))
        probs_pool = ctx.enter_context(tc.tile_pool(name="probs", bufs=11))
        gsum_pool = ctx.enter_context(tc.tile_pool(name="gsum", bufs=8))
        attn_pool = ctx.enter_context(tc.tile_pool(name="attnT", bufs=1))
        attn_c = [[attn_pool.tile([128, CH], BF, name=f"attn{h}_{ci}")
                   for ci in range(NCHA)] for h in range(HPC)]
        rden_pool = ctx.enter_context(tc.tile_pool(name="rden", bufs=2))
        out_pool = ctx.enter_context(tc.tile_pool(name="outstage", bufs=2))
        psS = ctx.enter_context(tc.tile_pool(name="psS", bufs=3, space="PSUM"))
        psAcc = ctx.enter_context(tc.tile_pool(name="psAcc", bufs=2, space="PSUM"))
        psD = ctx.enter_context(tc.tile_pool(name="psD", bufs=1, space="PSUM"))

        def phase_c(ci, groups, fine=False):
            # out[:, mo-group, chunk] = Wo_c.T @ attn  (8 mo per staged DMA;
            # fine=True DMAs per 2 mo so the final drain overlaps compute)
            w = CH
            sl = slice(ci * CH, (ci + 1) * CH)
            for g in groups:
                ob = out_pool.tile([128, 8, CH], BF, tag="ob")
                for gi in range(8):
                    mo = g * 8 + gi
                    po = psO.tile([128, CH], F32, tag="po")
                    for kk in range(HPC):
                        woff = (kk * MO + mo) * 128
                        nc.tensor.matmul(po[:, :w], wo_sb[:, woff:woff + 128],
                                         attn_c[kk][ci][:, :w],
                                         start=(kk == 0), stop=(kk == HPC - 1))
                    if gi % 2 == 0:
                        nc.scalar.copy(ob[:, gi, :w], po[:, :w])
                    else:
                        nc.vector.tensor_copy(ob[:, gi, :w], po[:, :w])
                    if fine and gi % 2 == 1:
                        nc.sync.dma_start(
                            outT[:, g * 8 + gi - 1:g * 8 + gi + 1, sl],
                            ob[:, gi - 1:gi + 1, :w])
                if not fine:
                    nc.sync.dma_start(outT[:, g * 8:(g + 1) * 8, sl],
                                      ob[:, :, :w])

        def phase_b(ci, h):
            tq0 = ci * CH
            w = CH
            jb = tq0 // 128
            nts = jb + w // 128
            pa = psAcc.tile([128, CH], F32, tag="pa")
            pd = psD.tile([128, CH], F32, tag="pd")

            pd_open = [False]

            def pd_mm(rhs, stop):
                nc.tensor.matmul(pd[:, :w], ones_sb[:], rhs,
                                 start=not pd_open[0], stop=stop)
                pd_open[0] = True

            # deferred-by-one pipeline: emit ss/exp for j, then math for j-1
            pend = [None]  # (j, pr, off)
            grp = []       # prob tiles awaiting group sum (ALL tiles join:
            # diag tiles are zero-filled left of `off` by a gpsimd memset, so
            # one ones-matmul covers 8 tiles, diag included)

            def flush_prev(last):
                if pend[0] is None:
                    return
                j, pr, off = pend[0]
                pend[0] = None
                nc.tensor.matmul(pa[:, off:w], v_c[j // 4][:, j % 4, :],
                                 pr[:, off:w],
                                 start=(j == 0), stop=(j == nts - 1))
                grp.append(pr)
                if len(grp) == 8 or last:
                    # pairwise tree-sum on DVE, one ones-matmul per group
                    lvl = list(grp)
                    while len(lvl) > 1:
                        nxt = []
                        for a in range(0, len(lvl) - 1, 2):
                            s = gsum_pool.tile([128, CH], BF, tag="g")
                            nc.vector.tensor_add(s[:, :w], lvl[a][:, :w],
                                                 lvl[a + 1][:, :w])
                            nxt.append(s)
                        if len(lvl) % 2:
                            nxt.append(lvl[-1])
                        lvl = nxt
                    pd_mm(lvl[0][:, :w], stop=last)
                    grp.clear()

            for j in range(nts):
                i = j - jb
                off = 0 if i < 1 else min(i * 128, w - 128)
                ss = psS.tile([128, CH], F32, tag="psS")
                nc.tensor.matmul(ss[:, off:w],
                                 qk_c[4][j // 4][:, (j % 4) * 128:(j % 4 + 1) * 128],
                                 qk_c[h][ci][:, off:w],
                                 start=True, stop=True)
                pr = probs_pool.tile([128, CH], BF, tag="probs")
                if off > 0:
                    # zero the left region on the idle gpsimd engine so this
                    # (masked) diag tile can join a denominator group-sum
                    nc.gpsimd.memset(pr[:, :off], 0.0)
                nc.scalar.activation(pr[:, off:w], ss[:, off:w], AF.Exp,
                                     scale=SCALING)
                if i >= 0:
                    nc.vector.tensor_mul(pr[:, off:w], pr[:, off:w],
                                         mask_sb[:, i * CH + off:i * CH + w])
                flush_prev(last=False)
                pend[0] = (j, pr, off)

            def fin():
                # deferred epilogue: emitted after the next interleaved
                # phase-C group so the tensor queue never HOL-blocks on the
                # vector tree-sum here
                flush_prev(last=True)
                rden = rden_pool.tile([128, CH], F32, tag="rden")
                nc.vector.reciprocal_approx_fast(out=rden[:, :w],
                                                 in_=pd[:, :w])
                nc.vector.tensor_mul(attn_c[h][ci][:, :w], pa[:, :w],
                                     rden[:, :w])
            return fin

        # Chunk order: mid/deep 512-chunks first (dense tensor work densifies
        # the A->B transition); the shallow chunk 0 lands at the end, its exps
        # hidden under interleaved C. ch3's transposes + RoPE (deferred from
        # finalize) interleave into the first B chunk: nothing there needs
        # them, and their engine work hides behind B's execution.
        with tc.tile_pool(name="psX", bufs=1, space="PSUM") as psX:
            fin = phase_b(1, 0)
            transposes(3, psX)
            fin()
            fin = phase_b(1, 1)
            rope_m(3, 0, tmp_pool2)
            rope_m(3, 1, tmp_pool2)
            fin()
            fin = phase_b(1, 2)
            rope_m(3, 2, tmp_pool2)
            rope_m(3, 3, tmp_pool2)
            fin()
            fin = phase_b(1, 3)
            rope_m(3, 4, tmp_pool2)
            fin()

        psO = ctx.enter_context(tc.tile_pool(name="psO", bufs=2, space="PSUM"))
        ORDER = [1, 2, 3, 0]
        for idx in range(1, len(ORDER)):
            ci = ORDER[idx]
            for h in range(HPC):
                fin = phase_b(ci, h)
                phase_c(ORDER[idx - 1], [h])
                fin()
        phase_c(ORDER[-1], [0, 1, 2, 3], fine=True)

    nc.compile()
    return nc


def get_nc():
    if "nc" not in _CACHE:
        _CACHE["nc"] = _build_nc()
    return _CACHE["nc"]


def _bf(x):
    return np.ascontiguousarray(np.asarray(x, dtype=np.float32).astype(BF16NP))


def prep_in_maps(hidden_states, cos, sin, Wq, Wk, Wv, Wo):
    # hidp[p, (ch*KO + ko)*ACH + t] = hidden[ch*ACH + t, ko*128 + p]
    hidT = np.asarray(hidden_states, dtype=np.float32).T  # [HID, T]
    hidp = _bf(hidT.reshape(KO, 128, NCHA, ACH).transpose(1, 2, 0, 3)
               .reshape(128, -1))
    cosT = _bf(np.asarray(cos).T)
    sinT_ = np.asarray(sin, dtype=np.float32).T.copy()
    sinT_[:D // 2, :] *= -1.0  # sign of rotate-half folded into sin table
    sinT = _bf(sinT_)
    # masksp[p, i*CH+f] = 1 if i*128+p <= f
    i_idx = np.arange(4)[None, :, None] * 128
    p_idx = np.arange(128)[:, None, None]
    f_idx = np.arange(CH)[None, None, :]
    masksp = _bf(((i_idx + p_idx) <= f_idx).astype(np.float32).reshape(128, -1))

    Wq = np.asarray(Wq, dtype=np.float32)
    Wk = np.asarray(Wk, dtype=np.float32)
    Wv = np.asarray(Wv, dtype=np.float32)
    Wo = np.asarray(Wo, dtype=np.float32)

    in_maps = []
    for c in range(NCORES):
        wqkv = np.concatenate([
            Wq[:, c * QW:(c + 1) * QW],
            Wk[:, c * D:(c + 1) * D],
            Wv[:, c * D:(c + 1) * D],
        ], axis=1)  # [HID, MW]
        # wqkvp[p, ko*MW+m] = wqkv[ko*128+p, m]
        wqkvp = _bf(wqkv.reshape(KO, 128, MW).transpose(1, 0, 2).reshape(128, -1))
        # wop[p, (kk*MO+mo)*128+q] = Wo[c*QW + kk*128+p, mo*128+q]
        wo_c = Wo[c * QW:(c + 1) * QW, :]
        wop_ = _bf(wo_c.reshape(HPC, 128, MO, 128).transpose(1, 0, 2, 3)
                   .reshape(128, -1))
        in_maps.append({
            "hidp": hidp,
            "wqkvp": wqkvp,
            "wop": wop_,
            "cosT": cosT,
            "sinT": sinT,
            "onesd": np.ones((128, 128), dtype=BF16NP),
            "identd": np.eye(128, dtype=np.float32).astype(BF16NP),
            "masksp": masksp,
        })
    return in_maps


def postprocess(results):
    # outT_p: [128, MO, T] bf16, out[t, mo*128+p] = outT_p[p, mo, t]
    acc = None
    for r in results:
        part = r["outT_p"].astype(np.float32)
        acc = part if acc is None else acc + part
    out = acc.transpose(1, 0, 2).reshape(HID, T)  # [HID, T]
    return np.ascontiguousarray(out.T).astype(np.float32)


def kernel(hidden_states, position_ids, cos, sin, Wq, Wk, Wv, Wo):
    from concourse.bass_utils import run_bass_kernel_spmd
    nc = get_nc()
    in_maps = prep_in_maps(hidden_states, cos, sin, Wq, Wk, Wv, Wo)
    res = run_bass_kernel_spmd(nc, in_maps, core_ids=list(range(NCORES)))
    return postprocess(res.results)
